# Initial kernel scaffold
#
"""Trainium2 Bass kernel for nn_Cosine_PredictingModule (GNN edge scoring).

Math (per edge e):
    heads = h_customer[src[e]]; tails = h_product[dst[e]]        (64-dim each)
    cos   = <heads, tails> / (||heads|| * ||tails||)             (eps-clamped)
    x     = relu(concat(heads, tails, cos) @ W1 + b1)            (129 -> 64)
    out   = sigmoid(x @ W2 + b2)                                 (64 -> 1)

Distribution: edges sharded contiguously across 8 cores (125k each); node
tables + weights replicated per core (SPMD, no collectives).

Device data path (per core):
  - Node tables are repacked host-side as fp16 rows of 128 elements:
    customer rows [raw(64) | normalized(64)], product rows [normalized | raw].
  - Edges are bucketed host-side by (src_chunk, dst_chunk) with 25k-row chunks
    so rebased indices fit the int16 index format of `dma_gather`.
  - Per bucket, two `dma_gather(transpose=True)` ops fetch rows feature-major:
    gathered tile partition p = row element p, free index = edge. No PE
    transposes needed.
  - DVE computes prod = norm_h * norm_t in place (into the tails tile's
    normalized half), then per 512-edge group three matmuls do the whole MLP:
      mm_h: [W1a; 0].T @ th          (raw head contribution)
      mm_t: [W1c_rep; W1b].T @ tt    (cos contribution via sum over prod + raw tails)
      mm2:  zero-padded W2 stack accumulating rows of a [16, 512] psum tile
  - ReLU alternates between ACT and DVE; sigmoid runs once per bucket.

Host: inverse-permutes bucket-sorted outputs, drops padding, concatenates.
"""

import sys

import numpy as np

sys.path.insert(0, "/opt/trn_rl_repo")

import concourse.bacc as bacc
import concourse.bass as bass
import concourse.tile as tile
from concourse import mybir
from concourse.bass_utils import run_bass_kernel_spmd

# Problem constants (hardcoded per contract).
N_CUST = 100000
N_PROD = 100000
N_EDGES = 1000000
D = 64
HIDDEN = 64

P = 128
ROWW = 128          # fp16 elems per packed table row (256B: dma_gather granule)
N_CORES = 8
E_CORE = N_EDGES // N_CORES  # 125000

CH = 25000          # table chunk rows (rebased idx < 32768 for int16)
NCH = 4             # chunks per table
NBKT = NCH * NCH    # (src_chunk, dst_chunk) buckets
BKCAP = 8192        # padded edges per bucket (mean 7812; overflow -> host spill)
E_PAD = NBKT * BKCAP

GRP = 512           # edges per PSUM group
NGRP = BKCAP // GRP  # 16 groups per bucket

F16 = mybir.dt.float16
F32 = mybir.dt.float32
I16 = mybir.dt.int16

IDXC = BKCAP // 16  # idx columns per bucket (16-partition wrap)


def build_program(nbkt=NBKT, repeat=1):
    """Build + compile the SPMD single-core program (same on all 8 cores).

    repeat>1 re-runs the whole bucket loop on the same data (for steady-state
    timing via deltas); outputs are simply overwritten with identical values.
    """
    nc = bacc.Bacc("TRN2", debug=False, target_bir_lowering=False,
                   num_devices=N_CORES)

    tab_c = nc.dram_tensor("tab_c", [N_CUST, ROWW], F16, kind="ExternalInput")
    tab_p = nc.dram_tensor("tab_p", [N_PROD, ROWW], F16, kind="ExternalInput")
    # int16 idx, 16-partition-wrapped and replicated to 128 partitions
    src_d = nc.dram_tensor("src16", [P, nbkt * IDXC], I16, kind="ExternalInput")
    dst_d = nc.dram_tensor("dst16", [P, nbkt * IDXC], I16, kind="ExternalInput")
    wh_d = nc.dram_tensor("wh", [P, HIDDEN], F16, kind="ExternalInput")
    wt_d = nc.dram_tensor("wt", [P, HIDDEN], F16, kind="ExternalInput")
    w2_d = nc.dram_tensor("w2s", [HIDDEN, NGRP * NGRP], F16, kind="ExternalInput")
    b1_d = nc.dram_tensor("b1c", [HIDDEN, 1], F32, kind="ExternalInput")
    b2_d = nc.dram_tensor("b2r", [NGRP, 1], F32, kind="ExternalInput")
    out_d = nc.dram_tensor("out", [nbkt * NGRP, GRP], F32, kind="ExternalOutput")

    from contextlib import ExitStack
    with tile.TileContext(nc) as tc, ExitStack() as ctx:
        const = ctx.enter_context(tc.tile_pool(name="const", bufs=1))
        p_gath = ctx.enter_context(tc.tile_pool(name="gath", bufs=2))
        p_idx = ctx.enter_context(tc.tile_pool(name="idx", bufs=3))
        p_xT = ctx.enter_context(tc.tile_pool(name="xT", bufs=4))
        p_out = ctx.enter_context(tc.tile_pool(name="outp", bufs=2))
        pp_h = ctx.enter_context(tc.tile_pool(name="ps_h", bufs=4, space="PSUM"))
        pp_o = ctx.enter_context(tc.tile_pool(name="ps_o", bufs=2, space="PSUM"))

        from concourse import library_config
        nc.gpsimd.load_library(library_config.mlp)

        wh = const.tile([P, HIDDEN], F16)
        wt = const.tile([P, HIDDEN], F16)
        w2 = const.tile([HIDDEN, NGRP * NGRP], F16)
        b1 = const.tile([HIDDEN, 1], F32)
        b2 = const.tile([NGRP, 1], F32)
        for t, d_ in ((wh, wh_d), (wt, wt_d), (w2, w2_d), (b1, b1_d),
                      (b2, b2_d)):
            nc.sync.dma_start(out=t[:], in_=d_[:])

        for it in range(nbkt * repeat):
            bkt = it % nbkt
            cs, cd = bkt // NCH, bkt % NCH
            sidx = p_idx.tile([P, IDXC], I16, tag="sidx")
            didx = p_idx.tile([P, IDXC], I16, tag="didx")
            nc.sync.dma_start(out=sidx[:], in_=src_d[:, bkt * IDXC:(bkt + 1) * IDXC])
            nc.sync.dma_start(out=didx[:], in_=dst_d[:, bkt * IDXC:(bkt + 1) * IDXC])

            # th/tt partitions: 0:64 normalized dims, 64:128 raw dims.
            # 4096-idx gathers hit the best measured ns/descriptor on the
            # single working SWDGE queue (multi-queue dma_gather corrupts
            # data on this runtime build).
            th = p_gath.tile([P, 1, BKCAP], F16, tag="th")
            tt = p_gath.tile([P, 1, BKCAP], F16, tag="tt")
            HK = BKCAP // 2
            HI = IDXC // 2
            for half in range(2):
                nc.gpsimd.dma_gather(
                    out_ap=th[:, :, half * HK:(half + 1) * HK],
                    in_ap=tab_c[cs * CH:(cs + 1) * CH, :],
                    idxs_ap=sidx[:, half * HI:(half + 1) * HI],
                    num_idxs=HK, num_idxs_reg=HK,
                    elem_size=ROWW, transpose=True, single_packet=False,
                )
                nc.gpsimd.dma_gather(
                    out_ap=tt[:, :, half * HK:(half + 1) * HK],
                    in_ap=tab_p[cd * CH:(cd + 1) * CH, :],
                    idxs_ap=didx[:, half * HI:(half + 1) * HI],
                    num_idxs=HK, num_idxs_reg=HK,
                    elem_size=ROWW, transpose=True, single_packet=False,
                )

            # prod = norm_h * norm_t, in place over tt's normalized half
            nc.vector.tensor_mul(
                out=tt[0:D, 0, :], in0=th[0:D, 0, :], in1=tt[0:D, 0, :])

            ps_o = pp_o.tile([NGRP, GRP], F32)
            out_sb = p_out.tile([NGRP, GRP], F32)

            for g in range(NGRP):
                sl = slice(g * GRP, (g + 1) * GRP)
                ps_h = pp_h.tile([HIDDEN, GRP], F32)
                nc.tensor.matmul(out=ps_h[:, :], lhsT=wh[:, :],
                                 rhs=th[:, 0, sl], start=True, stop=False)
                nc.tensor.matmul(out=ps_h[:, :], lhsT=wt[:, :],
                                 rhs=tt[:, 0, sl], start=False, stop=True)

                xT = p_xT.tile([HIDDEN, GRP], F16)
                if g % 2 == 0:
                    nc.scalar.activation(
                        out=xT[:, :], in_=ps_h[:, :],
                        func=mybir.ActivationFunctionType.Relu,
                        bias=b1[:, :])
                else:
                    nc.vector.tensor_scalar(
                        out=xT[:, :], in0=ps_h[:, :],
                        scalar1=b1[:, :], scalar2=0.0,
                        op0=mybir.AluOpType.add, op1=mybir.AluOpType.max)

                nc.tensor.matmul(out=ps_o[:, :],
                                 lhsT=w2[:, g * NGRP:(g + 1) * NGRP],
                                 rhs=xT[:, :],
                                 start=(g == 0), stop=(g == NGRP - 1))

            nc.scalar.activation(out=out_sb[:, :], in_=ps_o[:, :],
                                 func=mybir.ActivationFunctionType.Sigmoid,
                                 bias=b2[:, :])
            nc.sync.dma_start(out=out_d[bkt * NGRP:(bkt + 1) * NGRP, :],
                              in_=out_sb[:, :])

    nc.compile()
    return nc


def _pack_tables(h_customer, h_product):
    """-> (tab_c, tab_p) fp16 [N, 128] rows packed [normalized(64) | raw(64)]."""
    out = []
    for h in (h_customer, h_product):
        h = np.asarray(h, dtype=np.float32)
        norm = np.maximum(np.sqrt((h.astype(np.float64) ** 2).sum(axis=1)), 1e-12)
        tab = np.empty((h.shape[0], ROWW), dtype=np.float16)
        tab[:, :D] = (h / norm[:, None].astype(np.float32)).astype(np.float16)
        tab[:, D:] = h.astype(np.float16)
        out.append(tab)
    return out


def _w2_stack(W2):
    """[64, 1] -> [64, NGRP*NGRP] fp16 with W2 in column g*NGRP+g, zeros elsewhere."""
    w2 = np.asarray(W2, dtype=np.float32).reshape(HIDDEN)
    w2s = np.zeros((HIDDEN, NGRP, NGRP), dtype=np.float16)
    for g in range(NGRP):
        w2s[:, g, g] = w2.astype(np.float16)
    return w2s.reshape(HIDDEN, NGRP * NGRP)


def _wrap_idx16(idx_by_bucket):
    """list of [BKCAP] int16 arrays -> [128, NBKT*IDXC] wrapped + replicated.

    Each 4096-edge half-bucket is wrapped independently (it is its own
    dma_gather instruction on its own SWDGE queue).
    """
    cols = []
    for arr in idx_by_bucket:
        halves = [arr[:BKCAP // 2], arr[BKCAP // 2:]]
        w = np.concatenate([h.reshape(-1, 16).T for h in halves], axis=1)
        cols.append(np.tile(w, (8, 1)))  # replicate to 128 partitions
    return np.ascontiguousarray(np.concatenate(cols, axis=1))


def _bucketize(src, dst):
    """Sort one core's edges into (src_chunk, dst_chunk) buckets.

    Returns (src16_by_bucket, dst16_by_bucket, edge_pos, spill) where edge_pos
    maps each original edge to its padded position (-1 if spilled to host).
    """
    bucket = (src // CH) * NCH + (dst // CH)
    order = np.argsort(bucket, kind="stable")
    counts = np.bincount(bucket, minlength=NBKT)
    src16, dst16, spill = [], [], []
    edge_pos = np.full(src.shape[0], -1, dtype=np.int64)
    start = 0
    for b in range(NBKT):
        n = counts[b]
        take = min(n, BKCAP)
        idxs = order[start:start + take]
        if n > BKCAP:
            spill.extend(order[start + BKCAP:start + n].tolist())
        start += n
        s = np.zeros(BKCAP, dtype=np.int16)
        d_ = np.zeros(BKCAP, dtype=np.int16)
        s[:take] = (src[idxs] - (b // NCH) * CH).astype(np.int16)
        d_[:take] = (dst[idxs] - (b % NCH) * CH).astype(np.int16)
        edge_pos[idxs] = b * BKCAP + np.arange(take)
        src16.append(s)
        dst16.append(d_)
    return src16, dst16, edge_pos, np.asarray(spill, dtype=np.int64)


def _host_inputs(h_customer, h_product, src_idx, dst_idx, W1, b1, W2, b2):
    tab_c, tab_p = _pack_tables(h_customer, h_product)
    W1 = np.asarray(W1, dtype=np.float32)
    w1a = W1[:D]                    # heads block
    w1b = W1[D:2 * D]               # tails block
    w1c = W1[2 * D]                 # cos row, [64]
    wh = np.zeros((P, HIDDEN), dtype=np.float16)
    wh[D:] = w1a.astype(np.float16)
    wt = np.empty((P, HIDDEN), dtype=np.float16)
    wt[:D] = np.tile(w1c.astype(np.float16)[None, :], (D, 1))  # W1c replicated
    wt[D:] = w1b.astype(np.float16)
    w2s = _w2_stack(W2)
    b1c = np.asarray(b1, dtype=np.float32).reshape(HIDDEN, 1)
    b2r = np.full((NGRP, 1), np.float32(np.asarray(b2).reshape(-1)[0]))

    src_idx = np.asarray(src_idx).astype(np.int64).reshape(-1)
    dst_idx = np.asarray(dst_idx).astype(np.int64).reshape(-1)

    in_maps, metas = [], []
    for c in range(N_CORES):
        s = src_idx[c * E_CORE:(c + 1) * E_CORE]
        d_ = dst_idx[c * E_CORE:(c + 1) * E_CORE]
        src16, dst16, edge_pos, spill = _bucketize(s, d_)
        in_maps.append(dict(
            tab_c=tab_c, tab_p=tab_p,
            src16=_wrap_idx16(src16), dst16=_wrap_idx16(dst16),
            wh=wh, wt=wt, w2s=w2s, b1c=b1c, b2r=b2r,
        ))
        metas.append((edge_pos, spill))
    return in_maps, metas


def _np_reference_rows(h_c, h_p, src, dst, W1, b1, W2, b2):
    heads = np.asarray(h_c, np.float32)[src]
    tails = np.asarray(h_p, np.float32)[dst]
    hn = heads / np.maximum(np.linalg.norm(heads, axis=-1, keepdims=True), 1e-12)
    tn = tails / np.maximum(np.linalg.norm(tails, axis=-1, keepdims=True), 1e-12)
    cos = (hn * tn).sum(-1)
    cat = np.concatenate([heads, tails, cos[:, None]], axis=1)
    x = np.maximum(cat @ np.asarray(W1, np.float32) + np.asarray(b1, np.float32), 0)
    z = x @ np.asarray(W2, np.float32) + np.asarray(b2, np.float32)
    return (1.0 / (1.0 + np.exp(-z))).reshape(-1)


_PROG = None


def _get_program():
    global _PROG
    if _PROG is None:
        _PROG = build_program()
    return _PROG


def run(in_maps, trace=False, **kw):
    nc = _get_program()
    return run_bass_kernel_spmd(nc, in_maps, list(range(N_CORES)),
                                trace=trace, **kw)


def kernel(h_customer, h_product, src_idx, dst_idx, W1, b1, W2, b2):
    in_maps, metas = _host_inputs(h_customer, h_product, src_idx, dst_idx,
                                  W1, b1, W2, b2)
    res = run(in_maps).results

    src_idx = np.asarray(src_idx).astype(np.int64).reshape(-1)
    dst_idx = np.asarray(dst_idx).astype(np.int64).reshape(-1)
    out = np.empty(N_EDGES, dtype=np.float32)
    for c in range(N_CORES):
        flat = res[c]["out"].reshape(-1)
        edge_pos, spill = metas[c]
        ok = edge_pos >= 0
        seg = out[c * E_CORE:(c + 1) * E_CORE]
        seg[ok] = flat[edge_pos[ok]]
        if spill.size:  # bucket overflow: exact host computation for the rest
            gs = c * E_CORE + spill
            seg[spill] = _np_reference_rows(
                h_customer, h_product, src_idx[gs], dst_idx[gs],
                W1, b1, W2, b2)
    return out.reshape(N_EDGES, 1)



# revision 1
# speedup vs baseline: 1.1577x; 1.1577x over previous
"""Trainium2 Bass kernel for nn_Cosine_PredictingModule (GNN edge scoring).

Math (per edge e):
    heads = h_customer[src[e]]; tails = h_product[dst[e]]        (64-dim each)
    cos   = <heads, tails> / (||heads|| * ||tails||)             (eps-clamped)
    x     = relu(concat(heads, tails, cos) @ W1 + b1)            (129 -> 64)
    out   = sigmoid(x @ W2 + b2)                                 (64 -> 1)

Distribution: edges sharded contiguously across 8 cores (125k each); node
tables + weights replicated per core (SPMD, no collectives).

Device data path (per core):
  - Node tables are repacked host-side as fp16 rows of 128 elements:
    customer rows [raw(64) | normalized(64)], product rows [normalized | raw].
  - Edges are bucketed host-side by (src_chunk, dst_chunk) with 25k-row chunks
    so rebased indices fit the int16 index format of `dma_gather`.
  - Per bucket, two `dma_gather(transpose=True)` ops fetch rows feature-major:
    gathered tile partition p = row element p, free index = edge. No PE
    transposes needed.
  - DVE computes prod = norm_h * norm_t in place (into the tails tile's
    normalized half), then per 512-edge group three matmuls do the whole MLP:
      mm_h: [W1a; 0].T @ th          (raw head contribution)
      mm_t: [W1c_rep; W1b].T @ tt    (cos contribution via sum over prod + raw tails)
      mm2:  zero-padded W2 stack accumulating rows of a [16, 512] psum tile
  - ReLU alternates between ACT and DVE; sigmoid runs once per bucket.

Host: inverse-permutes bucket-sorted outputs, drops padding, concatenates.
"""

import sys

import numpy as np

sys.path.insert(0, "/opt/trn_rl_repo")

import concourse.bacc as bacc
import concourse.bass as bass
import concourse.tile as tile
from concourse import mybir
from concourse.bass_utils import run_bass_kernel_spmd

# Problem constants (hardcoded per contract).
N_CUST = 100000
N_PROD = 100000
N_EDGES = 1000000
D = 64
HIDDEN = 64

P = 128
ROWW = 128          # fp16 elems per packed table row (256B: dma_gather granule)
N_CORES = 8
E_CORE = N_EDGES // N_CORES  # 125000

CH = 25000          # table chunk rows (rebased idx < 32768 for int16)
NCH = 4             # chunks per table
NBKT = NCH * NCH    # (src_chunk, dst_chunk) buckets
BKCAP = 8192        # padded edges per bucket (mean 7812; overflow -> host spill)
E_PAD = NBKT * BKCAP

GRP = 512           # edges per PSUM group
NGRP = BKCAP // GRP  # 16 groups per bucket

F16 = mybir.dt.float16
F32 = mybir.dt.float32
I16 = mybir.dt.int16

IDXC = BKCAP // 16  # idx columns per bucket (16-partition wrap)


def build_program(nbkt=NBKT, repeat=1):
    """Build + compile the SPMD single-core program (same on all 8 cores).

    repeat>1 re-runs the whole bucket loop on the same data (for steady-state
    timing via deltas); outputs are simply overwritten with identical values.
    """
    nc = bacc.Bacc("TRN2", debug=False, target_bir_lowering=False,
                   num_devices=N_CORES)

    tab_c = nc.dram_tensor("tab_c", [N_CUST, ROWW], F16, kind="ExternalInput")
    tab_p = nc.dram_tensor("tab_p", [N_PROD, ROWW], F16, kind="ExternalInput")
    # int16 idx, 16-partition-wrapped and replicated to 128 partitions
    src_d = nc.dram_tensor("src16", [P, nbkt * IDXC], I16, kind="ExternalInput")
    dst_d = nc.dram_tensor("dst16", [P, nbkt * IDXC], I16, kind="ExternalInput")
    wh_d = nc.dram_tensor("wh", [P, HIDDEN], F16, kind="ExternalInput")
    wt_d = nc.dram_tensor("wt", [P, HIDDEN], F16, kind="ExternalInput")
    w2_d = nc.dram_tensor("w2s", [HIDDEN, NGRP * NGRP], F16, kind="ExternalInput")
    b1_d = nc.dram_tensor("b1c", [HIDDEN, 1], F32, kind="ExternalInput")
    b2_d = nc.dram_tensor("b2r", [NGRP, 1], F32, kind="ExternalInput")
    out_d = nc.dram_tensor("out", [nbkt * NGRP, GRP], F32, kind="ExternalOutput")

    from contextlib import ExitStack
    with tile.TileContext(nc) as tc, ExitStack() as ctx:
        const = ctx.enter_context(tc.tile_pool(name="const", bufs=1))
        p_gath = ctx.enter_context(tc.tile_pool(name="gath", bufs=2))
        p_idx = ctx.enter_context(tc.tile_pool(name="idx", bufs=3))
        p_xT = ctx.enter_context(tc.tile_pool(name="xT", bufs=4))
        p_out = ctx.enter_context(tc.tile_pool(name="outp", bufs=2))
        pp_h = ctx.enter_context(tc.tile_pool(name="ps_h", bufs=4, space="PSUM"))
        pp_o = ctx.enter_context(tc.tile_pool(name="ps_o", bufs=2, space="PSUM"))

        from concourse import library_config
        nc.gpsimd.load_library(library_config.mlp)

        wh = const.tile([P, HIDDEN], F16)
        wt = const.tile([P, HIDDEN], F16)
        w2 = const.tile([HIDDEN, NGRP * NGRP], F16)
        b1 = const.tile([HIDDEN, 1], F32)
        b2 = const.tile([NGRP, 1], F32)
        for t, d_ in ((wh, wh_d), (wt, wt_d), (w2, w2_d), (b1, b1_d),
                      (b2, b2_d)):
            nc.sync.dma_start(out=t[:], in_=d_[:])

        for it in range(nbkt * repeat):
            bkt = it % nbkt
            cs, cd = bkt // NCH, bkt % NCH
            sidx = p_idx.tile([P, IDXC], I16, tag="sidx")
            didx = p_idx.tile([P, IDXC], I16, tag="didx")
            nc.sync.dma_start(out=sidx[:], in_=src_d[:, bkt * IDXC:(bkt + 1) * IDXC])
            nc.sync.dma_start(out=didx[:], in_=dst_d[:, bkt * IDXC:(bkt + 1) * IDXC])

            # th/tt partitions: 0:64 normalized dims, 64:128 raw dims.
            # 4096-idx gathers hit the best measured ns/descriptor on the
            # single working SWDGE queue (multi-queue dma_gather corrupts
            # data on this runtime build).
            th = p_gath.tile([P, 1, BKCAP], F16, tag="th")
            tt = p_gath.tile([P, 1, BKCAP], F16, tag="tt")
            HK = BKCAP // 2
            HI = IDXC // 2
            for half in range(2):
                nc.gpsimd.dma_gather(
                    out_ap=th[:, :, half * HK:(half + 1) * HK],
                    in_ap=tab_c[cs * CH:(cs + 1) * CH, :],
                    idxs_ap=sidx[:, half * HI:(half + 1) * HI],
                    num_idxs=HK, num_idxs_reg=HK,
                    elem_size=ROWW, transpose=True, single_packet=False,
                )
                nc.gpsimd.dma_gather(
                    out_ap=tt[:, :, half * HK:(half + 1) * HK],
                    in_ap=tab_p[cd * CH:(cd + 1) * CH, :],
                    idxs_ap=didx[:, half * HI:(half + 1) * HI],
                    num_idxs=HK, num_idxs_reg=HK,
                    elem_size=ROWW, transpose=True, single_packet=False,
                )

            # prod = norm_h * norm_t, in place over tt's normalized half
            nc.vector.tensor_mul(
                out=tt[0:D, 0, :], in0=th[0:D, 0, :], in1=tt[0:D, 0, :])

            ps_o = pp_o.tile([NGRP, GRP], F32)
            out_sb = p_out.tile([NGRP, GRP], F32)

            for g in range(NGRP):
                sl = slice(g * GRP, (g + 1) * GRP)
                ps_h = pp_h.tile([HIDDEN, GRP], F32)
                nc.tensor.matmul(out=ps_h[:, :], lhsT=wh[:, :],
                                 rhs=th[:, 0, sl], start=True, stop=False)
                nc.tensor.matmul(out=ps_h[:, :], lhsT=wt[:, :],
                                 rhs=tt[:, 0, sl], start=False, stop=True)

                xT = p_xT.tile([HIDDEN, GRP], F16)
                if g % 2 == 0:
                    nc.scalar.activation(
                        out=xT[:, :], in_=ps_h[:, :],
                        func=mybir.ActivationFunctionType.Relu,
                        bias=b1[:, :])
                else:
                    nc.vector.tensor_scalar(
                        out=xT[:, :], in0=ps_h[:, :],
                        scalar1=b1[:, :], scalar2=0.0,
                        op0=mybir.AluOpType.add, op1=mybir.AluOpType.max)

                nc.tensor.matmul(out=ps_o[:, :],
                                 lhsT=w2[:, g * NGRP:(g + 1) * NGRP],
                                 rhs=xT[:, :],
                                 start=(g == 0), stop=(g == NGRP - 1))

            nc.scalar.activation(out=out_sb[:, :], in_=ps_o[:, :],
                                 func=mybir.ActivationFunctionType.Sigmoid,
                                 bias=b2[:, :])
            nc.sync.dma_start(out=out_d[bkt * NGRP:(bkt + 1) * NGRP, :],
                              in_=out_sb[:, :])

    nc.compile()
    return nc


def _pack_tables(h_customer, h_product):
    """-> (tab_c, tab_p) fp16 [N, 128] rows packed [normalized(64) | raw(64)]."""
    out = []
    for h in (h_customer, h_product):
        h = np.asarray(h, dtype=np.float32)
        norm = np.maximum(np.sqrt((h.astype(np.float64) ** 2).sum(axis=1)), 1e-12)
        tab = np.empty((h.shape[0], ROWW), dtype=np.float16)
        tab[:, :D] = (h / norm[:, None].astype(np.float32)).astype(np.float16)
        tab[:, D:] = h.astype(np.float16)
        out.append(tab)
    return out


def _w2_stack(W2):
    """[64, 1] -> [64, NGRP*NGRP] fp16 with W2 in column g*NGRP+g, zeros elsewhere."""
    w2 = np.asarray(W2, dtype=np.float32).reshape(HIDDEN)
    w2s = np.zeros((HIDDEN, NGRP, NGRP), dtype=np.float16)
    for g in range(NGRP):
        w2s[:, g, g] = w2.astype(np.float16)
    return w2s.reshape(HIDDEN, NGRP * NGRP)


def _wrap_idx16(idx_by_bucket):
    """list of [BKCAP] int16 arrays -> [128, NBKT*IDXC] wrapped + replicated.

    Each 4096-edge half-bucket is wrapped independently (it is its own
    dma_gather instruction on its own SWDGE queue).
    """
    cols = []
    for arr in idx_by_bucket:
        halves = [arr[:BKCAP // 2], arr[BKCAP // 2:]]
        w = np.concatenate([h.reshape(-1, 16).T for h in halves], axis=1)
        cols.append(np.tile(w, (8, 1)))  # replicate to 128 partitions
    return np.ascontiguousarray(np.concatenate(cols, axis=1))


def _bucketize(src, dst):
    """Sort one core's edges into (src_chunk, dst_chunk) buckets.

    Returns (src16_by_bucket, dst16_by_bucket, edge_pos, spill) where edge_pos
    maps each original edge to its padded position (-1 if spilled to host).
    """
    bucket = (src // CH) * NCH + (dst // CH)
    order = np.argsort(bucket, kind="stable")
    counts = np.bincount(bucket, minlength=NBKT)
    src16, dst16, spill = [], [], []
    edge_pos = np.full(src.shape[0], -1, dtype=np.int64)
    start = 0
    for b in range(NBKT):
        n = counts[b]
        take = min(n, BKCAP)
        idxs = order[start:start + take]
        if n > BKCAP:
            spill.extend(order[start + BKCAP:start + n].tolist())
        start += n
        s = np.zeros(BKCAP, dtype=np.int16)
        d_ = np.zeros(BKCAP, dtype=np.int16)
        s[:take] = (src[idxs] - (b // NCH) * CH).astype(np.int16)
        d_[:take] = (dst[idxs] - (b % NCH) * CH).astype(np.int16)
        edge_pos[idxs] = b * BKCAP + np.arange(take)
        src16.append(s)
        dst16.append(d_)
    return src16, dst16, edge_pos, np.asarray(spill, dtype=np.int64)


def _host_inputs(h_customer, h_product, src_idx, dst_idx, W1, b1, W2, b2):
    tab_c, tab_p = _pack_tables(h_customer, h_product)
    W1 = np.asarray(W1, dtype=np.float32)
    w1a = W1[:D]                    # heads block
    w1b = W1[D:2 * D]               # tails block
    w1c = W1[2 * D]                 # cos row, [64]
    wh = np.zeros((P, HIDDEN), dtype=np.float16)
    wh[D:] = w1a.astype(np.float16)
    wt = np.empty((P, HIDDEN), dtype=np.float16)
    wt[:D] = np.tile(w1c.astype(np.float16)[None, :], (D, 1))  # W1c replicated
    wt[D:] = w1b.astype(np.float16)
    w2s = _w2_stack(W2)
    b1c = np.asarray(b1, dtype=np.float32).reshape(HIDDEN, 1)
    b2r = np.full((NGRP, 1), np.float32(np.asarray(b2).reshape(-1)[0]))

    src_idx = np.asarray(src_idx).astype(np.int64).reshape(-1)
    dst_idx = np.asarray(dst_idx).astype(np.int64).reshape(-1)

    in_maps, metas = [], []
    for c in range(N_CORES):
        s = src_idx[c * E_CORE:(c + 1) * E_CORE]
        d_ = dst_idx[c * E_CORE:(c + 1) * E_CORE]
        src16, dst16, edge_pos, spill = _bucketize(s, d_)
        in_maps.append(dict(
            tab_c=tab_c, tab_p=tab_p,
            src16=_wrap_idx16(src16), dst16=_wrap_idx16(dst16),
            wh=wh, wt=wt, w2s=w2s, b1c=b1c, b2r=b2r,
        ))
        metas.append((edge_pos, spill))
    return in_maps, metas


def _np_reference_rows(h_c, h_p, src, dst, W1, b1, W2, b2):
    heads = np.asarray(h_c, np.float32)[src]
    tails = np.asarray(h_p, np.float32)[dst]
    hn = heads / np.maximum(np.linalg.norm(heads, axis=-1, keepdims=True), 1e-12)
    tn = tails / np.maximum(np.linalg.norm(tails, axis=-1, keepdims=True), 1e-12)
    cos = (hn * tn).sum(-1)
    cat = np.concatenate([heads, tails, cos[:, None]], axis=1)
    x = np.maximum(cat @ np.asarray(W1, np.float32) + np.asarray(b1, np.float32), 0)
    z = x @ np.asarray(W2, np.float32) + np.asarray(b2, np.float32)
    return (1.0 / (1.0 + np.exp(-z))).reshape(-1)


_PROG = None


def _get_program():
    global _PROG
    if _PROG is None:
        _PROG = build_program()
    return _PROG


def run(in_maps, trace=False, **kw):
    nc = _get_program()
    return run_bass_kernel_spmd(nc, in_maps, list(range(N_CORES)),
                                trace=trace, **kw)


def kernel(h_customer, h_product, src_idx, dst_idx, W1, b1, W2, b2):
    in_maps, metas = _host_inputs(h_customer, h_product, src_idx, dst_idx,
                                  W1, b1, W2, b2)
    res = run(in_maps).results

    src_idx = np.asarray(src_idx).astype(np.int64).reshape(-1)
    dst_idx = np.asarray(dst_idx).astype(np.int64).reshape(-1)
    out = np.empty(N_EDGES, dtype=np.float32)
    for c in range(N_CORES):
        flat = res[c]["out"].reshape(-1)
        edge_pos, spill = metas[c]
        ok = edge_pos >= 0
        seg = out[c * E_CORE:(c + 1) * E_CORE]
        seg[ok] = flat[edge_pos[ok]]
        if spill.size:  # bucket overflow: exact host computation for the rest
            gs = c * E_CORE + spill
            seg[spill] = _np_reference_rows(
                h_customer, h_product, src_idx[gs], dst_idx[gs],
                W1, b1, W2, b2)
    return out.reshape(N_EDGES, 1)

